# revision 38
# baseline (speedup 1.0000x reference)
"""GCN autoencoder (6x gcn_layer) on 8 TRN2 NeuronCores.

Strategy (v2):
  - Rows of adj_/X sharded across 8 cores; weights replicated; bf16 on
    device (fp32 PSUM), host does sharding / transposes / casts.
  - Reassociation: layers whose W *expands* width are computed as
    relu((A @ z) @ W) instead of relu(A @ (z W)) so the big adj-matmul
    always contracts against the narrower operand:
        l1: A@(X W1)    256 cols   (H-form, H1 local from replicated X)
        l2: A@(z1 W2)   256        (H-form)
        l3: A@(z2 W3)   128        (H-form)
        l4: (A@z3) W4   128        (z-form: gather z3, W4 deferred)
        l5: A@(z4 W5)   256        (H-form)
        l6: (A@z5) W6   256        (z-form: gather z5, W6 deferred)
    1280 adj-matmul columns/row-block vs 1664 unassociated (-23%), and
    the l4/l6 gathers shrink to 128/256 cols.
  - z-form carriers are emitted row-major by an identity-matmul
    transpose in the producing layer's epilogue (zT chunk @ I128).
  - adjT k-chunks 0:48 SBUF-resident (loaded once on the ACT DMA ring);
    chunks 48:64 streamed per layer per phase on the SP ring.
  - Per-phase production waves + balanced insert (as baseline): each
    phase's epilogue AllGathers the next layer's carrier; the consumer
    accumulates k-chunks in arrival-wave order.
  - Gather-dependent SBUF loads ride the ACT HWDGE ring so a pending
    AllGather can never FIFO-block the adj-stream/bounce DMAs (SP ring).
"""

import sys

import numpy as np

if "/opt/trn_rl_repo" not in sys.path:
    sys.path.insert(0, "/opt/trn_rl_repo")

import ml_dtypes

import concourse.bacc as bacc
import concourse.tile as tile
from concourse import mybir
from concourse.bass_utils import run_bass_kernel_spmd

N = 8192
D_IN = 512
NCORES = 8
R = N // NCORES  # 1024 rows per core
DIMS = [(512, 256), (256, 256), (256, 128), (128, 256), (256, 256), (256, 512)]

BF16 = mybir.dt.bfloat16
F32 = mybir.dt.float32
NP_BF16 = ml_dtypes.bfloat16
RELU = mybir.ActivationFunctionType.Relu

KO = N // 128  # 64 k-chunks over the gather dim
RT = R // 128  # 8 local row tiles
NPH = 2
PH = R // NPH  # 512 rows per phase
HALF = RT // NPH  # 4 k-chunks each core contributes per phase

NRES = 48  # adjT k-chunks SBUF-resident; KO-NRES streamed per layer
INSERT = 16  # phase-1 wave-0 chunks slotted in before the wave-1 join
# layers whose epilogues gather in half-phase chunks (both phases): at
# the small (mt=1) layers the consumer's PE time cannot cover a full
# phase-gather latency, so halved gathers land sooner. Empirically worse
# when applied to the mt=2 layers (extra serialized AG floors).
SPLIT_EPI = (2, 3)
JOINT = ()

# per-layer adj-matmul carrier width (cols) and form
CW = [256, 256, 128, 128, 256, 256]
ZFORM = [False, False, False, True, False, True]  # deferred-W layers
# z width out of each layer (after deferred W where applicable)
ZW = [256, 256, 128, 256, 256, 512]

_CACHED = {}


def _build():
    nc = bacc.Bacc(
        "TRN2",
        target_bir_lowering=False,
        debug=False,
        enable_asserts=False,
        num_devices=NCORES,
    )

    adjT = nc.dram_tensor("adjT", [N, R], BF16, kind="ExternalInput")
    h1_dram = nc.dram_tensor("H1", [N, DIMS[0][1]], BF16, kind="ExternalInput")
    w_dram = [
        nc.dram_tensor(f"W{i + 1}", list(DIMS[i]), BF16, kind="ExternalInput")
        for i in range(6)
    ]
    i_dram = nc.dram_tensor("I128", [128, 128], BF16, kind="ExternalInput")
    outT = nc.dram_tensor("outT", [DIMS[-1][1], R], F32, kind="ExternalOutput")

    adjT_r = adjT.ap().rearrange("(ko p) r -> p ko r", p=128)
    h1_r = h1_dram.ap().rearrange("(g p) d -> p g d", p=128)

    with tile.TileContext(nc) as tc:
        with (
            tc.tile_pool(name="adjres", bufs=1) as adjres_p,
            tc.tile_pool(name="adjstr", bufs=6) as adjstr_p,
            tc.tile_pool(name="wp", bufs=1) as wp,

            tc.tile_pool(name="cp", bufs=4) as cpool,
            tc.tile_pool(name="ztp", bufs=5) as ztp,
            tc.tile_pool(name="usb", bufs=2) as usbp,
            tc.tile_pool(name="hstage", bufs=4) as hstage,
            tc.tile_pool(name="ostage", bufs=3) as ostage,
            tc.tile_pool(name="psz", bufs=4, space="PSUM") as psz,
            tc.tile_pool(name="psu", bufs=2, space="PSUM") as psu,
            tc.tile_pool(name="psh", bufs=2, space="PSUM") as psh,
            tc.tile_pool(name="dram", bufs=1, space="DRAM") as dram,
        ):
            # ---- resident weights + identity ----
            w_sb = []
            for i, (di, do) in enumerate(DIMS):
                w_t = wp.tile([128, di // 128, do], BF16, name=f"w{i}_sb")
                nc.sync.dma_start(
                    w_t[:], w_dram[i].ap().rearrange("(kx p) n -> p kx n", p=128)
                )
                w_sb.append(w_t)
            i_sb = wp.tile([128, 128], BF16, name="i_sb")
            nc.sync.dma_start(i_sb[:], i_dram.ap())


            # warmup AllGather: absorbs the collective-stream first-use cost
            # (entry barrier + ncfw init) under layer 1's compute
            wu_in = dram.tile([16, 256], BF16, tag="wui", name="wui")
            wu_out = dram.tile([NCORES * 16, 256], BF16, addr_space="Shared",
                               tag="wuo", name="wuo")
            nc.gpsimd.collective_compute(
                "AllGather",
                mybir.AluOpType.bypass,
                ins=[wu_in[:].opt()],
                outs=[wu_out[:].opt()],
                replica_groups=[list(range(NCORES))],
            )

            # ---- C1 = H1 = X @ W1, precomputed on the host ----
            # two wave buffers [128, 32, 256]; chunk g -> C1[g//32][:, g%32]
            c_cur = [
                cpool.tile([128, KO // 2, 256], BF16, tag="c", name=f"c1_{w}")
                for w in range(NPH)
            ]
            # startup loads on the ACT ring, interleaved in consumption
            # order: C1 chunk g needed at ~g*0.52us, adjres ph0-half group
            # k (chunks 6k..6k+5) at ~3.1k us, C1 wave 1 from ~17us,
            # adjres ph1 halves only after l1's phase-0 (~58us+)
            adj_res = adjres_p.tile([128, NRES, R], BF16, name="adj_res")

            def _c1(w, q):
                nc.scalar.dma_start(
                    c_cur[w][:, q * 8 : q * 8 + 8, :],
                    h1_r[:, w * 32 + q * 8 : w * 32 + q * 8 + 8, :],
                )

            def _ares(q, half):
                lo, hi = q * 6, q * 6 + 6
                cl, ch = (0, PH) if half == 0 else (PH, R)
                nc.scalar.dma_start(
                    adj_res[:, lo:hi, cl:ch], adjT_r[:, lo:hi, cl:ch]
                )

            _c1(0, 0); _ares(0, 0); _c1(0, 1); _ares(1, 0)
            _c1(0, 2); _ares(2, 0); _c1(0, 3); _ares(3, 0)
            _ares(4, 0); _c1(1, 0); _ares(5, 0); _c1(1, 1)
            _ares(6, 0); _c1(1, 2); _ares(7, 0); _c1(1, 3)
            for q in range(8):
                _ares(q, 1)

            adj_stream_cache = {}

            def adj_mov(g, n):
                if g < NRES:
                    return adj_res[:, g, n * PH : (n + 1) * PH]
                grp = g // 2
                t = adj_stream_cache.get((grp, n))
                if t is None:
                    t = adjstr_p.tile([128, 2, PH], BF16, tag="adjs",
                                      name=f"as{grp}_{n}")
                    nc.sync.dma_start(
                        t[:], adjT_r[:, grp * 2 : grp * 2 + 2,
                                     n * PH : (n + 1) * PH]
                    )
                    adj_stream_cache[(grp, n)] = t
                return t[:, g % 2, :]

            def c1_read(m, g):
                return c_cur[g // (KO // 2)][:, g % (KO // 2),
                                             m * 128 : (m + 1) * 128]

            c_read = c1_read

            # consumption waves: layer 1 in production order (g ascending);
            # layers >=2 by producer phase ({c*8 + n*4 + j, j<4} per phase
            # n), with h=0 half-gather chunks first when the producer's
            # epilogue is split
            waves_l1 = [list(range(KO // 2)), list(range(KO // 2, KO))]

            def waves_for(li):
                if li == 0:
                    return waves_l1
                if (li - 1) in SPLIT_EPI:
                    # producer's gathers are split: each phase's h=0 chunks
                    # land first, so consume them first
                    return [
                        [c * RT + n * HALF + h * 2 + j
                         for h in range(2)
                         for c in range(NCORES) for j in range(2)]
                        for n in range(NPH)
                    ]
                return [
                    [c * RT + n * HALF + j
                     for c in range(NCORES) for j in range(HALF)]
                    for n in range(NPH)
                ]

            for li in range(6):
                di, do = DIMS[li]
                last = li == 5
                mt = CW[li] // 128          # adj-mm output width /128
                mtz = ZW[li] // 128         # z width /128
                kwaves = waves_for(li)
                adj_stream_cache.clear()

                if not last:
                    # next layer's carrier buffers (written by epilogue AGs)
                    c_next = [
                        cpool.tile([128, KO // 2, 256], BF16, tag="c",
                                   name=f"c{li + 2}_{w}")
                        for w in range(NPH)
                    ]
                    cw_next = CW[li + 1]

                    def make_reader(c_tiles, cwn):
                        def rd(m, g):
                            c, r8 = g // RT, g % RT
                            w, j = r8 // HALF, r8 % HALF
                            return c_tiles[w][:, c * HALF + j,
                                              m * 128 : (m + 1) * 128]
                        return rd

                ps_zs = [[psz.tile([128, PH], F32, tag="psz",
                                   name=f"psz{li}_{n}_{m}")
                          for m in range(mt)] for n in range(NPH)]
                mm_cnt = [[0] * mt for _ in range(NPH)]

                def emit_block(wb, n, lo=0, hi=None):
                    for g in kwaves[wb][lo:hi]:
                        mov = adj_mov(g, n)
                        for m in range(mt):
                            nc.tensor.matmul(
                                ps_zs[n][m][:],
                                c_read(m, g),
                                mov,
                                start=(mm_cnt[n][m] == 0),
                                stop=(mm_cnt[n][m] == KO - 1),
                            )
                            mm_cnt[n][m] += 1

                def emit_epilogue(n):
                    # ---- produce this phase's zT tiles ----
                    if ZFORM[li]:
                        # u = A @ C (unrelu'd); z = relu(u @ W_deferred)
                        u_sb = usbp.tile([128, 2, PH], BF16, tag="usb")
                        for m in range(mt):
                            nc.vector.tensor_copy(
                                u_sb[:, m, :], ps_zs[n][m][:]
                            )
                        zt_p = []
                        for mo in range(mtz):
                            ps_c = psu.tile([128, PH], F32, tag="psu")
                            for kx in range(mt):
                                nc.tensor.matmul(
                                    ps_c[:],
                                    w_sb[li][:, kx, mo * 128 : (mo + 1) * 128],
                                    u_sb[:, kx, :],
                                    start=(kx == 0),
                                    stop=(kx == mt - 1),
                                )
                            if last:
                                o_st = ostage.tile([128, PH], F32, tag="ost")
                                nc.scalar.activation(o_st[:], ps_c[:], RELU)
                                nc.sync.dma_start(
                                    outT[mo * 128 : (mo + 1) * 128,
                                         n * PH : (n + 1) * PH],
                                    o_st[:],
                                )
                            else:
                                z_t = ztp.tile([128, PH], BF16, tag="zt",
                                               name=f"z{li + 1}_{mo}_{n}")
                                nc.scalar.activation(z_t[:], ps_c[:], RELU)
                                zt_p.append(z_t)
                        if last:
                            return
                    else:
                        zt_p = []
                        for m in range(mt):
                            z_t = ztp.tile([128, PH], BF16, tag="zt",
                                           name=f"z{li + 1}_{m}_{n}")
                            nc.scalar.activation(z_t[:], ps_zs[n][m][:], RELU)
                            zt_p.append(z_t)

                    # ---- build C_{l+2}'s source rows: bounce + AllGather ----
                    # (optionally split in half-phase chunks so the consumer
                    # can start on the first half while the second gathers)
                    zform_next = ZFORM[li + 1]
                    do2 = ZW[li] if zform_next else DIMS[li + 1][1]
                    nhg = 2 if li in SPLIT_EPI else 1
                    jh = HALF // nhg
                    for h in range(nhg):
                        bounce = dram.tile(
                            [jh * 128, do2], BF16, tag=f"hb{li}_{n}_{h}",
                            name=f"hb{li}_{n}_{h}",
                        )
                        for j in range(h * jh, (h + 1) * jh):
                            ps_h = psh.tile([128, 256], F32, tag="psh")
                            if zform_next:
                                # row-major z via identity transpose:
                                # ps_h[:, co*128:...] = (zT[co][:, j])^T
                                for co in range(mtz):
                                    nc.tensor.matmul(
                                        ps_h[:, co * 128 : (co + 1) * 128],
                                        zt_p[co][:, j * 128 : (j + 1) * 128],
                                        i_sb[:],
                                        start=(co == 0),
                                        stop=(co == mtz - 1),
                                    )
                            else:
                                for kx in range(mtz):
                                    nc.tensor.matmul(
                                        ps_h[:, 0:do2],
                                        zt_p[kx][:, j * 128 : (j + 1) * 128],
                                        w_sb[li + 1][:, kx, :],
                                        start=(kx == 0),
                                        stop=(kx == mtz - 1),
                                    )
                            h_st = hstage.tile([128, 256], BF16, tag="hst")
                            nc.vector.tensor_copy(
                                h_st[:, 0:do2], ps_h[:, 0:do2]
                            )
                            # ACT ring: fires immediately (never queued
                            # behind the sync ring's stream prefetch)
                            nc.scalar.dma_start(
                                bounce[(j - h * jh) * 128
                                       : (j - h * jh + 1) * 128, :],
                                h_st[:, 0:do2],
                            )
                        gath = dram.tile(
                            [NCORES * jh * 128, do2], BF16,
                            addr_space="Shared",
                            tag=f"hg{li}_{n}_{h}", name=f"hg{li}_{n}_{h}",
                        )
                        nc.gpsimd.collective_compute(
                            "AllGather",
                            mybir.AluOpType.bypass,
                            ins=[bounce[:].opt()],
                            outs=[gath[:].opt()],
                            replica_groups=[list(range(NCORES))],
                        )
                        # gather-dependent loads on the SWDGE (gpsimd) ring
                        # (their AG-wait cannot FIFO-block either HWDGE
                        # ring); per core c so early chunks unblock first
                        g_r = gath.rearrange("(c j p) d -> p (c j) d",
                                             p=128, c=NCORES)
                        for c in range(NCORES):
                            nc.gpsimd.dma_start(
                                c_next[n][:, c * HALF + h * jh
                                          : c * HALF + (h + 1) * jh, 0:do2],
                                g_r[:, c * jh : (c + 1) * jh, :],
                            )

                if li in JOINT:
                    emit_block(0, 0)
                    emit_block(0, 1)
                    emit_block(1, 0)
                    emit_block(1, 1)
                    emit_epilogue(0)
                    emit_epilogue(1)
                else:
                    emit_block(0, 0)
                    insert = li > 0 and mt <= 2
                    if insert:
                        emit_block(0, 1, 0, INSERT)
                    emit_block(1, 0)
                    emit_epilogue(0)
                    emit_block(0, 1, INSERT if insert else 0, None)
                    emit_block(1, 1)
                    emit_epilogue(1)

                if not last:
                    c_read = make_reader(c_next, cw_next)

    nc.compile()
    return nc


def make_in_maps(inputs):
    X = np.asarray(inputs["X"], dtype=np.float32)
    adj = np.asarray(inputs["adj_"], dtype=np.float32)
    ws = [np.asarray(inputs[f"W{j + 1}"], np.float32).astype(NP_BF16)
          for j in range(6)]
    # H1 = X @ W1 on the host (0.008% of total FLOPs): removes the xT
    # stream and the XW1 tensor block from the device's critical startup
    h1 = (X @ np.asarray(inputs["W1"], dtype=np.float32)).astype(NP_BF16)
    eye = np.eye(128, dtype=NP_BF16)
    in_maps = []
    for i in range(NCORES):
        rows = slice(i * R, (i + 1) * R)
        m = {
            "adjT": np.ascontiguousarray(adj[rows, :].T).astype(NP_BF16),
            "H1": h1,
            "I128": eye,
        }
        for j in range(6):
            m[f"W{j + 1}"] = ws[j]
        in_maps.append(m)
    return in_maps


def kernel(**inputs):
    if "nc" not in _CACHED:
        _CACHED["nc"] = _build()
    nc = _CACHED["nc"]

    res = run_bass_kernel_spmd(nc, make_in_maps(inputs),
                               core_ids=list(range(NCORES)))
    out = np.concatenate(
        [np.asarray(r["outT"], dtype=np.float32).T for r in res.results], axis=0
    )
    return out


# revision 40
# speedup vs baseline: 1.0576x; 1.0576x over previous
"""GCN autoencoder (6x gcn_layer) on 8 TRN2 NeuronCores.

Strategy (v2):
  - Rows of adj_/X sharded across 8 cores; weights replicated; bf16 on
    device (fp32 PSUM), host does sharding / transposes / casts.
  - Reassociation: layers whose W *expands* width are computed as
    relu((A @ z) @ W) instead of relu(A @ (z W)) so the big adj-matmul
    always contracts against the narrower operand:
        l1: A@(X W1)    256 cols   (H-form, H1 local from replicated X)
        l2: A@(z1 W2)   256        (H-form)
        l3: A@(z2 W3)   128        (H-form)
        l4: (A@z3) W4   128        (z-form: gather z3, W4 deferred)
        l5: A@(z4 W5)   256        (H-form)
        l6: (A@z5) W6   256        (z-form: gather z5, W6 deferred)
    1280 adj-matmul columns/row-block vs 1664 unassociated (-23%), and
    the l4/l6 gathers shrink to 128/256 cols.
  - z-form carriers are emitted row-major by an identity-matmul
    transpose in the producing layer's epilogue (zT chunk @ I128).
  - adjT k-chunks 0:48 SBUF-resident (loaded once on the ACT DMA ring);
    chunks 48:64 streamed per layer per phase on the SP ring.
  - Per-phase production waves + balanced insert (as baseline): each
    phase's epilogue AllGathers the next layer's carrier; the consumer
    accumulates k-chunks in arrival-wave order. The small (mt=1) layers'
    gathers are additionally split in half-phase chunks (SPLIT_EPI).
  - Ring separation: bounce writes + resident/C1 loads on the ACT HWDGE
    ring, adj-stream/weights/out on the SP ring, gather-dependent C
    loads on the SWDGE (gpsimd) ring so an AllGather wait can never
    FIFO-block either HWDGE ring.
  - H1 = X @ W1 is precomputed on the host (0.008% of total FLOPs),
    removing the xT stream + XW1 block from the device's startup.
"""

import sys

import numpy as np

if "/opt/trn_rl_repo" not in sys.path:
    sys.path.insert(0, "/opt/trn_rl_repo")

import ml_dtypes

import concourse.bacc as bacc
import concourse.tile as tile
from concourse import mybir
from concourse.bass_utils import run_bass_kernel_spmd

N = 8192
D_IN = 512
NCORES = 8
R = N // NCORES  # 1024 rows per core
DIMS = [(512, 256), (256, 256), (256, 128), (128, 256), (256, 256), (256, 512)]

BF16 = mybir.dt.bfloat16
F32 = mybir.dt.float32
NP_BF16 = ml_dtypes.bfloat16
RELU = mybir.ActivationFunctionType.Relu

KO = N // 128  # 64 k-chunks over the gather dim
RT = R // 128  # 8 local row tiles
NPH = 2
PH = R // NPH  # 512 rows per phase
HALF = RT // NPH  # 4 k-chunks each core contributes per phase

NRES = 48  # adjT k-chunks SBUF-resident; KO-NRES streamed per layer
INSERT = 16  # phase-1 wave-0 chunks slotted in before the wave-1 join
# layers whose epilogues gather in half-phase chunks (both phases): at
# the small (mt=1) layers the consumer's PE time cannot cover a full
# phase-gather latency, so halved gathers land sooner. Empirically worse
# when applied to the mt=2 layers (extra serialized AG floors).
SPLIT_EPI = (2, 3)
JOINT = ()

# per-layer adj-matmul carrier width (cols) and form
CW = [256, 256, 128, 128, 256, 256]
ZFORM = [False, False, False, True, False, True]  # deferred-W layers
# z width out of each layer (after deferred W where applicable)
ZW = [256, 256, 128, 256, 256, 512]

_CACHED = {}


def _build():
    nc = bacc.Bacc(
        "TRN2",
        target_bir_lowering=False,
        debug=False,
        enable_asserts=False,
        num_devices=NCORES,
    )

    adjT = nc.dram_tensor("adjT", [N, R], BF16, kind="ExternalInput")
    h1_dram = nc.dram_tensor("H1", [N, DIMS[0][1]], BF16, kind="ExternalInput")
    w_dram = [
        nc.dram_tensor(f"W{i + 1}", list(DIMS[i]), BF16, kind="ExternalInput")
        for i in range(6)
    ]
    i_dram = nc.dram_tensor("I128", [128, 128], BF16, kind="ExternalInput")
    outT = nc.dram_tensor("outT", [DIMS[-1][1], R], F32, kind="ExternalOutput")

    adjT_r = adjT.ap().rearrange("(ko p) r -> p ko r", p=128)
    h1_r = h1_dram.ap().rearrange("(g p) d -> p g d", p=128)

    with tile.TileContext(nc) as tc:
        with (
            tc.tile_pool(name="adjres", bufs=1) as adjres_p,
            tc.tile_pool(name="adjstr", bufs=6) as adjstr_p,
            tc.tile_pool(name="wp", bufs=1) as wp,
            tc.tile_pool(name="cp", bufs=4) as cpool,
            tc.tile_pool(name="ztp", bufs=5) as ztp,
            tc.tile_pool(name="usb", bufs=2) as usbp,
            tc.tile_pool(name="hstage", bufs=4) as hstage,
            tc.tile_pool(name="ostage", bufs=3) as ostage,
            tc.tile_pool(name="psz", bufs=4, space="PSUM") as psz,
            tc.tile_pool(name="psu", bufs=2, space="PSUM") as psu,
            tc.tile_pool(name="psh", bufs=2, space="PSUM") as psh,
            tc.tile_pool(name="dram", bufs=1, space="DRAM") as dram,
        ):
            # ---- resident weights + identity ----
            w_sb = []
            for i, (di, do) in enumerate(DIMS):
                w_t = wp.tile([128, di // 128, do], BF16, name=f"w{i}_sb")
                nc.sync.dma_start(
                    w_t[:], w_dram[i].ap().rearrange("(kx p) n -> p kx n", p=128)
                )
                w_sb.append(w_t)
            i_sb = wp.tile([128, 128], BF16, name="i_sb")
            nc.sync.dma_start(i_sb[:], i_dram.ap())


            # warmup AllGather: absorbs the collective-stream first-use cost
            # (entry barrier + ncfw init) under layer 1's compute
            wu_in = dram.tile([16, 256], BF16, tag="wui", name="wui")
            wu_out = dram.tile([NCORES * 16, 256], BF16, addr_space="Shared",
                               tag="wuo", name="wuo")
            nc.gpsimd.collective_compute(
                "AllGather",
                mybir.AluOpType.bypass,
                ins=[wu_in[:].opt()],
                outs=[wu_out[:].opt()],
                replica_groups=[list(range(NCORES))],
            )

            # ---- C1 = H1 = X @ W1, precomputed on the host ----
            # two wave buffers [128, 32, 256]; chunk g -> C1[g//32][:, g%32]
            c_cur = [
                cpool.tile([128, KO // 2, 256], BF16, tag="c", name=f"c1_{w}")
                for w in range(NPH)
            ]
            # startup loads on the ACT ring, interleaved in consumption
            # order: C1 chunk g needed at ~g*0.52us, adjres ph0-half group
            # k (chunks 6k..6k+5) at ~3.1k us, C1 wave 1 from ~17us,
            # adjres ph1 halves only after l1's phase-0 (~58us+)
            adj_res = adjres_p.tile([128, NRES, R], BF16, name="adj_res")

            def _c1(w, q):
                nc.scalar.dma_start(
                    c_cur[w][:, q * 8 : q * 8 + 8, :],
                    h1_r[:, w * 32 + q * 8 : w * 32 + q * 8 + 8, :],
                )

            def _ares(q, half):
                lo, hi = q * 6, q * 6 + 6
                cl, ch = (0, PH) if half == 0 else (PH, R)
                nc.scalar.dma_start(
                    adj_res[:, lo:hi, cl:ch], adjT_r[:, lo:hi, cl:ch]
                )

            _c1(0, 0); _ares(0, 0); _c1(0, 1); _ares(1, 0)
            _c1(0, 2); _ares(2, 0); _c1(0, 3); _ares(3, 0)
            _ares(4, 0); _c1(1, 0); _ares(5, 0); _c1(1, 1)
            _ares(6, 0); _c1(1, 2); _ares(7, 0); _c1(1, 3)
            for q in range(8):
                _ares(q, 1)

            adj_stream_cache = {}

            def adj_mov(g, n):
                if g < NRES:
                    return adj_res[:, g, n * PH : (n + 1) * PH]
                grp = g // 2
                t = adj_stream_cache.get((grp, n))
                if t is None:
                    t = adjstr_p.tile([128, 2, PH], BF16, tag="adjs",
                                      name=f"as{grp}_{n}")
                    nc.sync.dma_start(
                        t[:], adjT_r[:, grp * 2 : grp * 2 + 2,
                                     n * PH : (n + 1) * PH]
                    )
                    adj_stream_cache[(grp, n)] = t
                return t[:, g % 2, :]

            def c1_read(m, g):
                return c_cur[g // (KO // 2)][:, g % (KO // 2),
                                             m * 128 : (m + 1) * 128]

            c_read = c1_read

            # consumption waves: layer 1 in production order (g ascending);
            # layers >=2 by producer phase ({c*8 + n*4 + j, j<4} per phase
            # n), with h=0 half-gather chunks first when the producer's
            # epilogue is split
            waves_l1 = [list(range(KO // 2)), list(range(KO // 2, KO))]

            def waves_for(li):
                if li == 0:
                    return waves_l1
                if (li - 1) in SPLIT_EPI:
                    # producer's gathers are split: each phase's h=0 chunks
                    # land first, so consume them first
                    return [
                        [c * RT + n * HALF + h * 2 + j
                         for h in range(2)
                         for c in range(NCORES) for j in range(2)]
                        for n in range(NPH)
                    ]
                return [
                    [c * RT + n * HALF + j
                     for c in range(NCORES) for j in range(HALF)]
                    for n in range(NPH)
                ]

            for li in range(6):
                di, do = DIMS[li]
                last = li == 5
                mt = CW[li] // 128          # adj-mm output width /128
                mtz = ZW[li] // 128         # z width /128
                kwaves = waves_for(li)
                adj_stream_cache.clear()

                if not last:
                    # next layer's carrier buffers (written by epilogue AGs)
                    c_next = [
                        cpool.tile([128, KO // 2, 256], BF16, tag="c",
                                   name=f"c{li + 2}_{w}")
                        for w in range(NPH)
                    ]
                    cw_next = CW[li + 1]

                    def make_reader(c_tiles, cwn):
                        def rd(m, g):
                            c, r8 = g // RT, g % RT
                            w, j = r8 // HALF, r8 % HALF
                            return c_tiles[w][:, c * HALF + j,
                                              m * 128 : (m + 1) * 128]
                        return rd

                ps_zs = [[psz.tile([128, PH], F32, tag="psz",
                                   name=f"psz{li}_{n}_{m}")
                          for m in range(mt)] for n in range(NPH)]
                mm_cnt = [[0] * mt for _ in range(NPH)]

                def emit_block(wb, n, lo=0, hi=None):
                    for g in kwaves[wb][lo:hi]:
                        mov = adj_mov(g, n)
                        for m in range(mt):
                            nc.tensor.matmul(
                                ps_zs[n][m][:],
                                c_read(m, g),
                                mov,
                                start=(mm_cnt[n][m] == 0),
                                stop=(mm_cnt[n][m] == KO - 1),
                            )
                            mm_cnt[n][m] += 1

                def emit_epilogue(n):
                    # ---- produce this phase's zT tiles ----
                    if ZFORM[li]:
                        # u = A @ C (unrelu'd); z = relu(u @ W_deferred)
                        u_sb = usbp.tile([128, 2, PH], BF16, tag="usb")
                        for m in range(mt):
                            nc.vector.tensor_copy(
                                u_sb[:, m, :], ps_zs[n][m][:]
                            )
                        zt_p = []
                        for mo in range(mtz):
                            ps_c = psu.tile([128, PH], F32, tag="psu")
                            for kx in range(mt):
                                nc.tensor.matmul(
                                    ps_c[:],
                                    w_sb[li][:, kx, mo * 128 : (mo + 1) * 128],
                                    u_sb[:, kx, :],
                                    start=(kx == 0),
                                    stop=(kx == mt - 1),
                                )
                            if last:
                                o_st = ostage.tile([128, PH], F32, tag="ost")
                                nc.scalar.activation(o_st[:], ps_c[:], RELU)
                                nc.sync.dma_start(
                                    outT[mo * 128 : (mo + 1) * 128,
                                         n * PH : (n + 1) * PH],
                                    o_st[:],
                                )
                            else:
                                z_t = ztp.tile([128, PH], BF16, tag="zt",
                                               name=f"z{li + 1}_{mo}_{n}")
                                nc.scalar.activation(z_t[:], ps_c[:], RELU)
                                zt_p.append(z_t)
                        if last:
                            return
                    else:
                        zt_p = []
                        for m in range(mt):
                            z_t = ztp.tile([128, PH], BF16, tag="zt",
                                           name=f"z{li + 1}_{m}_{n}")
                            nc.scalar.activation(z_t[:], ps_zs[n][m][:], RELU)
                            zt_p.append(z_t)

                    # ---- build C_{l+2}'s source rows: bounce + AllGather ----
                    # (optionally split in half-phase chunks so the consumer
                    # can start on the first half while the second gathers)
                    zform_next = ZFORM[li + 1]
                    do2 = ZW[li] if zform_next else DIMS[li + 1][1]
                    nhg = 2 if li in SPLIT_EPI else 1
                    jh = HALF // nhg
                    for h in range(nhg):
                        bounce = dram.tile(
                            [jh * 128, do2], BF16, tag=f"hb{li}_{n}_{h}",
                            name=f"hb{li}_{n}_{h}",
                        )
                        for j in range(h * jh, (h + 1) * jh):
                            ps_h = psh.tile([128, 256], F32, tag="psh")
                            if zform_next:
                                # row-major z via identity transpose:
                                # ps_h[:, co*128:...] = (zT[co][:, j])^T
                                for co in range(mtz):
                                    nc.tensor.matmul(
                                        ps_h[:, co * 128 : (co + 1) * 128],
                                        zt_p[co][:, j * 128 : (j + 1) * 128],
                                        i_sb[:],
                                        start=(co == 0),
                                        stop=(co == mtz - 1),
                                    )
                            else:
                                for kx in range(mtz):
                                    nc.tensor.matmul(
                                        ps_h[:, 0:do2],
                                        zt_p[kx][:, j * 128 : (j + 1) * 128],
                                        w_sb[li + 1][:, kx, :],
                                        start=(kx == 0),
                                        stop=(kx == mtz - 1),
                                    )
                            h_st = hstage.tile([128, 256], BF16, tag="hst")
                            nc.vector.tensor_copy(
                                h_st[:, 0:do2], ps_h[:, 0:do2]
                            )
                            # ACT ring: fires immediately (never queued
                            # behind the sync ring's stream prefetch)
                            nc.scalar.dma_start(
                                bounce[(j - h * jh) * 128
                                       : (j - h * jh + 1) * 128, :],
                                h_st[:, 0:do2],
                            )
                        gath = dram.tile(
                            [NCORES * jh * 128, do2], BF16,
                            addr_space="Shared",
                            tag=f"hg{li}_{n}_{h}", name=f"hg{li}_{n}_{h}",
                        )
                        nc.gpsimd.collective_compute(
                            "AllGather",
                            mybir.AluOpType.bypass,
                            ins=[bounce[:].opt()],
                            outs=[gath[:].opt()],
                            replica_groups=[list(range(NCORES))],
                        )
                        # gather-dependent loads on the SWDGE (gpsimd) ring
                        # (their AG-wait cannot FIFO-block either HWDGE
                        # ring); per core c so early chunks unblock first
                        g_r = gath.rearrange("(c j p) d -> p (c j) d",
                                             p=128, c=NCORES)
                        for c in range(NCORES):
                            nc.gpsimd.dma_start(
                                c_next[n][:, c * HALF + h * jh
                                          : c * HALF + (h + 1) * jh, 0:do2],
                                g_r[:, c * jh : (c + 1) * jh, :],
                            )

                if li in JOINT:
                    emit_block(0, 0)
                    emit_block(0, 1)
                    emit_block(1, 0)
                    emit_block(1, 1)
                    emit_epilogue(0)
                    emit_epilogue(1)
                else:
                    emit_block(0, 0)
                    insert = li > 0 and mt <= 2
                    if insert:
                        emit_block(0, 1, 0, INSERT)
                    emit_block(1, 0)
                    emit_epilogue(0)
                    emit_block(0, 1, INSERT if insert else 0, None)
                    emit_block(1, 1)
                    emit_epilogue(1)

                if not last:
                    c_read = make_reader(c_next, cw_next)

    nc.compile()
    return nc


def make_in_maps(inputs):
    X = np.asarray(inputs["X"], dtype=np.float32)
    adj = np.asarray(inputs["adj_"], dtype=np.float32)
    ws = [np.asarray(inputs[f"W{j + 1}"], np.float32).astype(NP_BF16)
          for j in range(6)]
    # H1 = X @ W1 on the host (0.008% of total FLOPs): removes the xT
    # stream and the XW1 tensor block from the device's critical startup
    h1 = (X @ np.asarray(inputs["W1"], dtype=np.float32)).astype(NP_BF16)
    eye = np.eye(128, dtype=NP_BF16)
    in_maps = []
    for i in range(NCORES):
        rows = slice(i * R, (i + 1) * R)
        m = {
            "adjT": np.ascontiguousarray(adj[rows, :].T).astype(NP_BF16),
            "H1": h1,
            "I128": eye,
        }
        for j in range(6):
            m[f"W{j + 1}"] = ws[j]
        in_maps.append(m)
    return in_maps


def kernel(**inputs):
    if "nc" not in _CACHED:
        _CACHED["nc"] = _build()
    nc = _CACHED["nc"]

    res = run_bass_kernel_spmd(nc, make_in_maps(inputs),
                               core_ids=list(range(NCORES)))
    out = np.concatenate(
        [np.asarray(r["outT"], dtype=np.float32).T for r in res.results], axis=0
    )
    return out



# revision 42
# speedup vs baseline: 1.0751x; 1.0166x over previous
"""GCN autoencoder (6x gcn_layer) on 8 TRN2 NeuronCores.

Strategy (v2):
  - Rows of adj_/X sharded across 8 cores; weights replicated; bf16 on
    device (fp32 PSUM), host does sharding / transposes / casts.
  - Reassociation: layers whose W *expands* width are computed as
    relu((A @ z) @ W) instead of relu(A @ (z W)) so the big adj-matmul
    always contracts against the narrower operand:
        l1: A@(X W1)    256 cols   (H-form, H1 local from replicated X)
        l2: A@(z1 W2)   256        (H-form)
        l3: A@(z2 W3)   128        (H-form)
        l4: (A@z3) W4   128        (z-form: gather z3, W4 deferred)
        l5: A@(z4 W5)   256        (H-form)
        l6: (A@z5) W6   256        (z-form: gather z5, W6 deferred)
    1280 adj-matmul columns/row-block vs 1664 unassociated (-23%), and
    the l4/l6 gathers shrink to 128/256 cols.
  - z-form carriers are emitted row-major by an identity-matmul
    transpose in the producing layer's epilogue (zT chunk @ I128).
  - adjT k-chunks 0:48 SBUF-resident (loaded once on the ACT DMA ring);
    chunks 48:64 streamed per layer per phase on the SP ring.
  - Per-phase production waves + balanced insert (as baseline): each
    phase's epilogue AllGathers the next layer's carrier; the consumer
    accumulates k-chunks in arrival-wave order. The small (mt=1) layers'
    gathers are additionally split in half-phase chunks (SPLIT_EPI).
  - Ring separation: bounce writes + resident/C1 loads on the ACT HWDGE
    ring, adj-stream/weights/out on the SP ring, gather-dependent C
    loads on the SWDGE (gpsimd) ring so an AllGather wait can never
    FIFO-block either HWDGE ring.
  - H1 = X @ W1 is precomputed on the host (0.008% of total FLOPs),
    removing the xT stream + XW1 block from the device's startup.
"""

import sys

import numpy as np

if "/opt/trn_rl_repo" not in sys.path:
    sys.path.insert(0, "/opt/trn_rl_repo")

import ml_dtypes

import concourse.bacc as bacc
import concourse.tile as tile
from concourse import mybir
from concourse.bass_utils import run_bass_kernel_spmd

N = 8192
D_IN = 512
NCORES = 8
R = N // NCORES  # 1024 rows per core
DIMS = [(512, 256), (256, 256), (256, 128), (128, 256), (256, 256), (256, 512)]

BF16 = mybir.dt.bfloat16
F32 = mybir.dt.float32
NP_BF16 = ml_dtypes.bfloat16
RELU = mybir.ActivationFunctionType.Relu

KO = N // 128  # 64 k-chunks over the gather dim
RT = R // 128  # 8 local row tiles
NPH = 2
PH = R // NPH  # 512 rows per phase
HALF = RT // NPH  # 4 k-chunks each core contributes per phase

NRES = 48  # adjT k-chunks SBUF-resident; KO-NRES streamed per layer
INSERT = 16  # phase-1 wave-0 chunks slotted in before the wave-1 join
# layers whose epilogues gather in half-phase chunks (both phases): at
# the small (mt=1) layers the consumer's PE time cannot cover a full
# phase-gather latency, so halved gathers land sooner. Empirically worse
# when applied to the mt=2 layers (extra serialized AG floors).
SPLIT_EPI = (2, 3)
JOINT = ()

# per-layer adj-matmul carrier width (cols) and form
CW = [256, 256, 128, 128, 256, 256]
ZFORM = [False, False, False, True, False, True]  # deferred-W layers
# z width out of each layer (after deferred W where applicable)
ZW = [256, 256, 128, 256, 256, 512]

_CACHED = {}


def _build():
    nc = bacc.Bacc(
        "TRN2",
        target_bir_lowering=False,
        debug=False,
        enable_asserts=False,
        num_devices=NCORES,
    )

    adjT = nc.dram_tensor("adjT", [N, R], BF16, kind="ExternalInput")
    h1_dram = nc.dram_tensor("H1", [N, DIMS[0][1]], BF16, kind="ExternalInput")
    w_dram = [
        nc.dram_tensor(f"W{i + 1}", list(DIMS[i]), BF16, kind="ExternalInput")
        for i in range(6)
    ]
    i_dram = nc.dram_tensor("I128", [128, 128], BF16, kind="ExternalInput")
    outT = nc.dram_tensor("outT", [DIMS[-1][1], R], F32, kind="ExternalOutput")

    adjT_r = adjT.ap().rearrange("(ko p) r -> p ko r", p=128)
    h1_r = h1_dram.ap().rearrange("(g p) d -> p g d", p=128)

    with tile.TileContext(nc) as tc:
        with (
            tc.tile_pool(name="adjres", bufs=1) as adjres_p,
            tc.tile_pool(name="adjstr", bufs=6) as adjstr_p,
            tc.tile_pool(name="wp", bufs=1) as wp,
            tc.tile_pool(name="cp", bufs=4) as cpool,
            tc.tile_pool(name="ztp", bufs=5) as ztp,
            tc.tile_pool(name="usb", bufs=2) as usbp,
            tc.tile_pool(name="hstage", bufs=4) as hstage,
            tc.tile_pool(name="ostage", bufs=3) as ostage,
            tc.tile_pool(name="psz", bufs=4, space="PSUM") as psz,
            tc.tile_pool(name="psu", bufs=2, space="PSUM") as psu,
            tc.tile_pool(name="psh", bufs=2, space="PSUM") as psh,
            tc.tile_pool(name="dram", bufs=1, space="DRAM") as dram,
        ):
            # ---- resident weights + identity ----
            w_sb = []
            for i, (di, do) in enumerate(DIMS):
                w_t = wp.tile([128, di // 128, do], BF16, name=f"w{i}_sb")
                nc.sync.dma_start(
                    w_t[:], w_dram[i].ap().rearrange("(kx p) n -> p kx n", p=128)
                )
                w_sb.append(w_t)
            i_sb = wp.tile([128, 128], BF16, name="i_sb")
            nc.sync.dma_start(i_sb[:], i_dram.ap())


            # warmup AllGather: absorbs the collective-stream first-use cost
            # (entry barrier + ncfw init) under layer 1's compute
            wu_in = dram.tile([16, 256], BF16, tag="wui", name="wui")
            wu_out = dram.tile([NCORES * 16, 256], BF16, addr_space="Shared",
                               tag="wuo", name="wuo")
            nc.gpsimd.collective_compute(
                "AllGather",
                mybir.AluOpType.bypass,
                ins=[wu_in[:].opt()],
                outs=[wu_out[:].opt()],
                replica_groups=[list(range(NCORES))],
            )

            # ---- C1 = H1 = X @ W1, precomputed on the host ----
            # two wave buffers [128, 32, 256]; chunk g -> C1[g//32][:, g%32]
            c_cur = [
                cpool.tile([128, KO // 2, 256], BF16, tag="c", name=f"c1_{w}")
                for w in range(NPH)
            ]
            # startup loads on the ACT ring, interleaved in consumption
            # order: C1 chunk g needed at ~g*0.52us, adjres ph0-half group
            # k (chunks 6k..6k+5) at ~3.1k us, C1 wave 1 from ~17us,
            # adjres ph1 halves only after l1's phase-0 (~58us+)
            adj_res = adjres_p.tile([128, NRES, R], BF16, name="adj_res")

            def _c1(w, q):
                nc.scalar.dma_start(
                    c_cur[w][:, q * 8 : q * 8 + 8, :],
                    h1_r[:, w * 32 + q * 8 : w * 32 + q * 8 + 8, :],
                )

            def _ares(q, half):
                lo, hi = q * 6, q * 6 + 6
                cl, ch = (0, PH) if half == 0 else (PH, R)
                nc.scalar.dma_start(
                    adj_res[:, lo:hi, cl:ch], adjT_r[:, lo:hi, cl:ch]
                )

            _c1(0, 0); _ares(0, 0); _c1(0, 1); _ares(1, 0)
            _c1(0, 2); _ares(2, 0); _c1(0, 3); _ares(3, 0)
            _ares(4, 0); _c1(1, 0); _ares(5, 0); _c1(1, 1)
            _ares(6, 0); _c1(1, 2); _ares(7, 0); _c1(1, 3)
            for q in range(8):
                _ares(q, 1)

            adj_stream_cache = {}

            def adj_mov(g, n):
                if g < NRES:
                    return adj_res[:, g, n * PH : (n + 1) * PH]
                grp = g // 2
                t = adj_stream_cache.get((grp, n))
                if t is None:
                    t = adjstr_p.tile([128, 2, PH], BF16, tag="adjs",
                                      name=f"as{grp}_{n}")
                    nc.sync.dma_start(
                        t[:], adjT_r[:, grp * 2 : grp * 2 + 2,
                                     n * PH : (n + 1) * PH]
                    )
                    adj_stream_cache[(grp, n)] = t
                return t[:, g % 2, :]

            def c1_read(m, g):
                return c_cur[g // (KO // 2)][:, g % (KO // 2),
                                             m * 128 : (m + 1) * 128]

            c_read = c1_read

            # consumption waves: layer 1 in production order (g ascending);
            # layers >=2 by producer phase ({c*8 + n*4 + j, j<4} per phase
            # n), with h=0 half-gather chunks first when the producer's
            # epilogue is split
            waves_l1 = [list(range(KO // 2)), list(range(KO // 2, KO))]

            def waves_for(li):
                if li == 0:
                    return waves_l1
                w0 = [c * RT + j for c in range(NCORES) for j in range(HALF)]
                if (li - 1) in SPLIT_EPI:
                    # producer's phase-0 gather is split: its h=0 chunks
                    # land first, so consume them first
                    w0 = [c * RT + h * 2 + j
                          for h in range(2)
                          for c in range(NCORES) for j in range(2)]
                w1 = [c * RT + HALF + j
                      for c in range(NCORES) for j in range(HALF)]
                return [w0, w1]

            for li in range(6):
                di, do = DIMS[li]
                last = li == 5
                mt = CW[li] // 128          # adj-mm output width /128
                mtz = ZW[li] // 128         # z width /128
                kwaves = waves_for(li)
                adj_stream_cache.clear()

                if not last:
                    # next layer's carrier buffers (written by epilogue AGs)
                    c_next = [
                        cpool.tile([128, KO // 2, 256], BF16, tag="c",
                                   name=f"c{li + 2}_{w}")
                        for w in range(NPH)
                    ]
                    cw_next = CW[li + 1]

                    def make_reader(c_tiles, cwn):
                        def rd(m, g):
                            c, r8 = g // RT, g % RT
                            w, j = r8 // HALF, r8 % HALF
                            return c_tiles[w][:, c * HALF + j,
                                              m * 128 : (m + 1) * 128]
                        return rd

                ps_zs = [[psz.tile([128, PH], F32, tag="psz",
                                   name=f"psz{li}_{n}_{m}")
                          for m in range(mt)] for n in range(NPH)]
                mm_cnt = [[0] * mt for _ in range(NPH)]

                def emit_block(wb, n, lo=0, hi=None):
                    for g in kwaves[wb][lo:hi]:
                        mov = adj_mov(g, n)
                        for m in range(mt):
                            nc.tensor.matmul(
                                ps_zs[n][m][:],
                                c_read(m, g),
                                mov,
                                start=(mm_cnt[n][m] == 0),
                                stop=(mm_cnt[n][m] == KO - 1),
                            )
                            mm_cnt[n][m] += 1

                def emit_epilogue(n):
                    # ---- produce this phase's zT tiles ----
                    if ZFORM[li]:
                        # u = A @ C (unrelu'd); z = relu(u @ W_deferred)
                        u_sb = usbp.tile([128, 2, PH], BF16, tag="usb")
                        for m in range(mt):
                            nc.vector.tensor_copy(
                                u_sb[:, m, :], ps_zs[n][m][:]
                            )
                        zt_p = []
                        for mo in range(mtz):
                            ps_c = psu.tile([128, PH], F32, tag="psu")
                            for kx in range(mt):
                                nc.tensor.matmul(
                                    ps_c[:],
                                    w_sb[li][:, kx, mo * 128 : (mo + 1) * 128],
                                    u_sb[:, kx, :],
                                    start=(kx == 0),
                                    stop=(kx == mt - 1),
                                )
                            if last:
                                o_st = ostage.tile([128, PH], F32, tag="ost")
                                nc.scalar.activation(o_st[:], ps_c[:], RELU)
                                nc.sync.dma_start(
                                    outT[mo * 128 : (mo + 1) * 128,
                                         n * PH : (n + 1) * PH],
                                    o_st[:],
                                )
                            else:
                                z_t = ztp.tile([128, PH], BF16, tag="zt",
                                               name=f"z{li + 1}_{mo}_{n}")
                                nc.scalar.activation(z_t[:], ps_c[:], RELU)
                                zt_p.append(z_t)
                        if last:
                            return
                    else:
                        zt_p = []
                        for m in range(mt):
                            z_t = ztp.tile([128, PH], BF16, tag="zt",
                                           name=f"z{li + 1}_{m}_{n}")
                            nc.scalar.activation(z_t[:], ps_zs[n][m][:], RELU)
                            zt_p.append(z_t)

                    # ---- build C_{l+2}'s source rows: bounce + AllGather ----
                    # (optionally split in half-phase chunks so the consumer
                    # can start on the first half while the second gathers)
                    zform_next = ZFORM[li + 1]
                    do2 = ZW[li] if zform_next else DIMS[li + 1][1]
                    nhg = 2 if (li in SPLIT_EPI and n == 0) else 1
                    jh = HALF // nhg
                    for h in range(nhg):
                        bounce = dram.tile(
                            [jh * 128, do2], BF16, tag=f"hb{li}_{n}_{h}",
                            name=f"hb{li}_{n}_{h}",
                        )
                        for j in range(h * jh, (h + 1) * jh):
                            ps_h = psh.tile([128, 256], F32, tag="psh")
                            if zform_next:
                                # row-major z via identity transpose:
                                # ps_h[:, co*128:...] = (zT[co][:, j])^T
                                for co in range(mtz):
                                    nc.tensor.matmul(
                                        ps_h[:, co * 128 : (co + 1) * 128],
                                        zt_p[co][:, j * 128 : (j + 1) * 128],
                                        i_sb[:],
                                        start=(co == 0),
                                        stop=(co == mtz - 1),
                                    )
                            else:
                                for kx in range(mtz):
                                    nc.tensor.matmul(
                                        ps_h[:, 0:do2],
                                        zt_p[kx][:, j * 128 : (j + 1) * 128],
                                        w_sb[li + 1][:, kx, :],
                                        start=(kx == 0),
                                        stop=(kx == mtz - 1),
                                    )
                            h_st = hstage.tile([128, 256], BF16, tag="hst")
                            nc.vector.tensor_copy(
                                h_st[:, 0:do2], ps_h[:, 0:do2]
                            )
                            # ACT ring: fires immediately (never queued
                            # behind the sync ring's stream prefetch)
                            nc.scalar.dma_start(
                                bounce[(j - h * jh) * 128
                                       : (j - h * jh + 1) * 128, :],
                                h_st[:, 0:do2],
                            )
                        gath = dram.tile(
                            [NCORES * jh * 128, do2], BF16,
                            addr_space="Shared",
                            tag=f"hg{li}_{n}_{h}", name=f"hg{li}_{n}_{h}",
                        )
                        nc.gpsimd.collective_compute(
                            "AllGather",
                            mybir.AluOpType.bypass,
                            ins=[bounce[:].opt()],
                            outs=[gath[:].opt()],
                            replica_groups=[list(range(NCORES))],
                        )
                        # gather-dependent loads on the SWDGE (gpsimd) ring
                        # (their AG-wait cannot FIFO-block either HWDGE
                        # ring); per core c so early chunks unblock first
                        g_r = gath.rearrange("(c j p) d -> p (c j) d",
                                             p=128, c=NCORES)
                        for c in range(NCORES):
                            nc.gpsimd.dma_start(
                                c_next[n][:, c * HALF + h * jh
                                          : c * HALF + (h + 1) * jh, 0:do2],
                                g_r[:, c * jh : (c + 1) * jh, :],
                            )

                if li in JOINT:
                    emit_block(0, 0)
                    emit_block(0, 1)
                    emit_block(1, 0)
                    emit_block(1, 1)
                    emit_epilogue(0)
                    emit_epilogue(1)
                else:
                    emit_block(0, 0)
                    insert = li > 0 and mt <= 2
                    if insert:
                        emit_block(0, 1, 0, INSERT)
                    emit_block(1, 0)
                    emit_epilogue(0)
                    emit_block(0, 1, INSERT if insert else 0, None)
                    emit_block(1, 1)
                    emit_epilogue(1)

                if not last:
                    c_read = make_reader(c_next, cw_next)

    nc.compile()
    return nc


def make_in_maps(inputs):
    X = np.asarray(inputs["X"], dtype=np.float32)
    adj = np.asarray(inputs["adj_"], dtype=np.float32)
    ws = [np.asarray(inputs[f"W{j + 1}"], np.float32).astype(NP_BF16)
          for j in range(6)]
    # H1 = X @ W1 on the host (0.008% of total FLOPs): removes the xT
    # stream and the XW1 tensor block from the device's critical startup
    h1 = (X @ np.asarray(inputs["W1"], dtype=np.float32)).astype(NP_BF16)
    eye = np.eye(128, dtype=NP_BF16)
    in_maps = []
    for i in range(NCORES):
        rows = slice(i * R, (i + 1) * R)
        m = {
            "adjT": np.ascontiguousarray(adj[rows, :].T).astype(NP_BF16),
            "H1": h1,
            "I128": eye,
        }
        for j in range(6):
            m[f"W{j + 1}"] = ws[j]
        in_maps.append(m)
    return in_maps


def kernel(**inputs):
    if "nc" not in _CACHED:
        _CACHED["nc"] = _build()
    nc = _CACHED["nc"]

    res = run_bass_kernel_spmd(nc, make_in_maps(inputs),
                               core_ids=list(range(NCORES)))
    out = np.concatenate(
        [np.asarray(r["outT"], dtype=np.float32).T for r in res.results], axis=0
    )
    return out



# revision 43
# speedup vs baseline: 1.0769x; 1.0016x over previous
"""GCN autoencoder (6x gcn_layer) on 8 TRN2 NeuronCores.

Strategy (v2):
  - Rows of adj_/X sharded across 8 cores; weights replicated; bf16 on
    device (fp32 PSUM), host does sharding / transposes / casts.
  - Reassociation: layers whose W *expands* width are computed as
    relu((A @ z) @ W) instead of relu(A @ (z W)) so the big adj-matmul
    always contracts against the narrower operand:
        l1: A@(X W1)    256 cols   (H-form, H1 local from replicated X)
        l2: A@(z1 W2)   256        (H-form)
        l3: A@(z2 W3)   128        (H-form)
        l4: (A@z3) W4   128        (z-form: gather z3, W4 deferred)
        l5: A@(z4 W5)   256        (H-form)
        l6: (A@z5) W6   256        (z-form: gather z5, W6 deferred)
    1280 adj-matmul columns/row-block vs 1664 unassociated (-23%), and
    the l4/l6 gathers shrink to 128/256 cols.
  - z-form carriers are emitted row-major by an identity-matmul
    transpose in the producing layer's epilogue (zT chunk @ I128).
  - adjT k-chunks 0:48 SBUF-resident (loaded once on the ACT DMA ring);
    chunks 48:64 streamed per layer per phase on the SP ring.
  - Per-phase production waves + balanced insert (as baseline): each
    phase's epilogue AllGathers the next layer's carrier; the consumer
    accumulates k-chunks in arrival-wave order. The small (mt=1) layers'
    gathers are additionally split in half-phase chunks (SPLIT_EPI).
  - Ring separation: bounce writes + resident/C1 loads on the ACT HWDGE
    ring, adj-stream/weights/out on the SP ring, gather-dependent C
    loads on the SWDGE (gpsimd) ring so an AllGather wait can never
    FIFO-block either HWDGE ring.
  - H1 = X @ W1 is precomputed on the host (0.008% of total FLOPs),
    removing the xT stream + XW1 block from the device's startup.
"""

import sys

import numpy as np

if "/opt/trn_rl_repo" not in sys.path:
    sys.path.insert(0, "/opt/trn_rl_repo")

import ml_dtypes

import concourse.bacc as bacc
import concourse.tile as tile
from concourse import mybir
from concourse.bass_utils import run_bass_kernel_spmd

N = 8192
D_IN = 512
NCORES = 8
R = N // NCORES  # 1024 rows per core
DIMS = [(512, 256), (256, 256), (256, 128), (128, 256), (256, 256), (256, 512)]

BF16 = mybir.dt.bfloat16
F32 = mybir.dt.float32
NP_BF16 = ml_dtypes.bfloat16
RELU = mybir.ActivationFunctionType.Relu

KO = N // 128  # 64 k-chunks over the gather dim
RT = R // 128  # 8 local row tiles
NPH = 2
PH = R // NPH  # 512 rows per phase
HALF = RT // NPH  # 4 k-chunks each core contributes per phase

NRES = 48  # adjT k-chunks SBUF-resident; KO-NRES streamed per layer
INSERT = 16  # phase-1 wave-0 chunks slotted in before the wave-1 join
# layers whose epilogues gather in half-phase chunks (both phases): at
# the small (mt=1) layers the consumer's PE time cannot cover a full
# phase-gather latency, so halved gathers land sooner. Empirically worse
# when applied to the mt=2 layers (extra serialized AG floors).
SPLIT_EPI = (2, 3, 4)
JOINT = ()

# per-layer adj-matmul carrier width (cols) and form
CW = [256, 256, 128, 128, 256, 256]
ZFORM = [False, False, False, True, False, True]  # deferred-W layers
# z width out of each layer (after deferred W where applicable)
ZW = [256, 256, 128, 256, 256, 512]

_CACHED = {}


def _build():
    nc = bacc.Bacc(
        "TRN2",
        target_bir_lowering=False,
        debug=False,
        enable_asserts=False,
        num_devices=NCORES,
    )

    adjT = nc.dram_tensor("adjT", [N, R], BF16, kind="ExternalInput")
    h1_dram = nc.dram_tensor("H1", [N, DIMS[0][1]], BF16, kind="ExternalInput")
    w_dram = [
        nc.dram_tensor(f"W{i + 1}", list(DIMS[i]), BF16, kind="ExternalInput")
        for i in range(6)
    ]
    i_dram = nc.dram_tensor("I128", [128, 128], BF16, kind="ExternalInput")
    outT = nc.dram_tensor("outT", [DIMS[-1][1], R], F32, kind="ExternalOutput")

    adjT_r = adjT.ap().rearrange("(ko p) r -> p ko r", p=128)
    h1_r = h1_dram.ap().rearrange("(g p) d -> p g d", p=128)

    with tile.TileContext(nc) as tc:
        with (
            tc.tile_pool(name="adjres", bufs=1) as adjres_p,
            tc.tile_pool(name="adjstr", bufs=6) as adjstr_p,
            tc.tile_pool(name="wp", bufs=1) as wp,
            tc.tile_pool(name="cp", bufs=4) as cpool,
            tc.tile_pool(name="ztp", bufs=5) as ztp,
            tc.tile_pool(name="usb", bufs=2) as usbp,
            tc.tile_pool(name="hstage", bufs=4) as hstage,
            tc.tile_pool(name="ostage", bufs=3) as ostage,
            tc.tile_pool(name="psz", bufs=4, space="PSUM") as psz,
            tc.tile_pool(name="psu", bufs=2, space="PSUM") as psu,
            tc.tile_pool(name="psh", bufs=2, space="PSUM") as psh,
            tc.tile_pool(name="dram", bufs=1, space="DRAM") as dram,
        ):
            # ---- resident weights + identity ----
            w_sb = []
            for i, (di, do) in enumerate(DIMS):
                w_t = wp.tile([128, di // 128, do], BF16, name=f"w{i}_sb")
                nc.sync.dma_start(
                    w_t[:], w_dram[i].ap().rearrange("(kx p) n -> p kx n", p=128)
                )
                w_sb.append(w_t)
            i_sb = wp.tile([128, 128], BF16, name="i_sb")
            nc.sync.dma_start(i_sb[:], i_dram.ap())


            # warmup AllGather: absorbs the collective-stream first-use cost
            # (entry barrier + ncfw init) under layer 1's compute
            wu_in = dram.tile([16, 256], BF16, tag="wui", name="wui")
            wu_out = dram.tile([NCORES * 16, 256], BF16, addr_space="Shared",
                               tag="wuo", name="wuo")
            nc.gpsimd.collective_compute(
                "AllGather",
                mybir.AluOpType.bypass,
                ins=[wu_in[:].opt()],
                outs=[wu_out[:].opt()],
                replica_groups=[list(range(NCORES))],
            )

            # ---- C1 = H1 = X @ W1, precomputed on the host ----
            # two wave buffers [128, 32, 256]; chunk g -> C1[g//32][:, g%32]
            c_cur = [
                cpool.tile([128, KO // 2, 256], BF16, tag="c", name=f"c1_{w}")
                for w in range(NPH)
            ]
            # startup loads on the ACT ring, interleaved in consumption
            # order: C1 chunk g needed at ~g*0.52us, adjres ph0-half group
            # k (chunks 6k..6k+5) at ~3.1k us, C1 wave 1 from ~17us,
            # adjres ph1 halves only after l1's phase-0 (~58us+)
            adj_res = adjres_p.tile([128, NRES, R], BF16, name="adj_res")

            def _c1(w, q):
                nc.scalar.dma_start(
                    c_cur[w][:, q * 8 : q * 8 + 8, :],
                    h1_r[:, w * 32 + q * 8 : w * 32 + q * 8 + 8, :],
                )

            def _ares(q, half):
                lo, hi = q * 6, q * 6 + 6
                cl, ch = (0, PH) if half == 0 else (PH, R)
                nc.scalar.dma_start(
                    adj_res[:, lo:hi, cl:ch], adjT_r[:, lo:hi, cl:ch]
                )

            _c1(0, 0); _ares(0, 0); _c1(0, 1); _ares(1, 0)
            _c1(0, 2); _ares(2, 0); _c1(0, 3); _ares(3, 0)
            _ares(4, 0); _c1(1, 0); _ares(5, 0); _c1(1, 1)
            _ares(6, 0); _c1(1, 2); _ares(7, 0); _c1(1, 3)
            for q in range(8):
                _ares(q, 1)

            adj_stream_cache = {}

            def adj_mov(g, n):
                if g < NRES:
                    return adj_res[:, g, n * PH : (n + 1) * PH]
                grp = g // 2
                t = adj_stream_cache.get((grp, n))
                if t is None:
                    t = adjstr_p.tile([128, 2, PH], BF16, tag="adjs",
                                      name=f"as{grp}_{n}")
                    nc.sync.dma_start(
                        t[:], adjT_r[:, grp * 2 : grp * 2 + 2,
                                     n * PH : (n + 1) * PH]
                    )
                    adj_stream_cache[(grp, n)] = t
                return t[:, g % 2, :]

            def c1_read(m, g):
                return c_cur[g // (KO // 2)][:, g % (KO // 2),
                                             m * 128 : (m + 1) * 128]

            c_read = c1_read

            # consumption waves: layer 1 in production order (g ascending);
            # layers >=2 by producer phase ({c*8 + n*4 + j, j<4} per phase
            # n), with h=0 half-gather chunks first when the producer's
            # epilogue is split
            waves_l1 = [list(range(KO // 2)), list(range(KO // 2, KO))]

            def waves_for(li):
                if li == 0:
                    return waves_l1
                w0 = [c * RT + j for c in range(NCORES) for j in range(HALF)]
                if (li - 1) in SPLIT_EPI:
                    # producer's phase-0 gather is split: its h=0 chunks
                    # land first, so consume them first
                    w0 = [c * RT + h * 2 + j
                          for h in range(2)
                          for c in range(NCORES) for j in range(2)]
                w1 = [c * RT + HALF + j
                      for c in range(NCORES) for j in range(HALF)]
                return [w0, w1]

            for li in range(6):
                di, do = DIMS[li]
                last = li == 5
                mt = CW[li] // 128          # adj-mm output width /128
                mtz = ZW[li] // 128         # z width /128
                kwaves = waves_for(li)
                adj_stream_cache.clear()

                if not last:
                    # next layer's carrier buffers (written by epilogue AGs)
                    c_next = [
                        cpool.tile([128, KO // 2, 256], BF16, tag="c",
                                   name=f"c{li + 2}_{w}")
                        for w in range(NPH)
                    ]
                    cw_next = CW[li + 1]

                    def make_reader(c_tiles, cwn):
                        def rd(m, g):
                            c, r8 = g // RT, g % RT
                            w, j = r8 // HALF, r8 % HALF
                            return c_tiles[w][:, c * HALF + j,
                                              m * 128 : (m + 1) * 128]
                        return rd

                ps_zs = [[psz.tile([128, PH], F32, tag="psz",
                                   name=f"psz{li}_{n}_{m}")
                          for m in range(mt)] for n in range(NPH)]
                mm_cnt = [[0] * mt for _ in range(NPH)]

                def emit_block(wb, n, lo=0, hi=None):
                    for g in kwaves[wb][lo:hi]:
                        mov = adj_mov(g, n)
                        for m in range(mt):
                            nc.tensor.matmul(
                                ps_zs[n][m][:],
                                c_read(m, g),
                                mov,
                                start=(mm_cnt[n][m] == 0),
                                stop=(mm_cnt[n][m] == KO - 1),
                            )
                            mm_cnt[n][m] += 1

                def emit_epilogue(n):
                    # ---- produce this phase's zT tiles ----
                    if ZFORM[li]:
                        # u = A @ C (unrelu'd); z = relu(u @ W_deferred)
                        u_sb = usbp.tile([128, 2, PH], BF16, tag="usb")
                        for m in range(mt):
                            nc.vector.tensor_copy(
                                u_sb[:, m, :], ps_zs[n][m][:]
                            )
                        zt_p = []
                        for mo in range(mtz):
                            ps_c = psu.tile([128, PH], F32, tag="psu")
                            for kx in range(mt):
                                nc.tensor.matmul(
                                    ps_c[:],
                                    w_sb[li][:, kx, mo * 128 : (mo + 1) * 128],
                                    u_sb[:, kx, :],
                                    start=(kx == 0),
                                    stop=(kx == mt - 1),
                                )
                            if last:
                                o_st = ostage.tile([128, PH], F32, tag="ost")
                                nc.scalar.activation(o_st[:], ps_c[:], RELU)
                                nc.sync.dma_start(
                                    outT[mo * 128 : (mo + 1) * 128,
                                         n * PH : (n + 1) * PH],
                                    o_st[:],
                                )
                            else:
                                z_t = ztp.tile([128, PH], BF16, tag="zt",
                                               name=f"z{li + 1}_{mo}_{n}")
                                nc.scalar.activation(z_t[:], ps_c[:], RELU)
                                zt_p.append(z_t)
                        if last:
                            return
                    else:
                        zt_p = []
                        for m in range(mt):
                            z_t = ztp.tile([128, PH], BF16, tag="zt",
                                           name=f"z{li + 1}_{m}_{n}")
                            nc.scalar.activation(z_t[:], ps_zs[n][m][:], RELU)
                            zt_p.append(z_t)

                    # ---- build C_{l+2}'s source rows: bounce + AllGather ----
                    # (optionally split in half-phase chunks so the consumer
                    # can start on the first half while the second gathers)
                    zform_next = ZFORM[li + 1]
                    do2 = ZW[li] if zform_next else DIMS[li + 1][1]
                    nhg = 2 if (li in SPLIT_EPI and n == 0) else 1
                    jh = HALF // nhg
                    for h in range(nhg):
                        bounce = dram.tile(
                            [jh * 128, do2], BF16, tag=f"hb{li}_{n}_{h}",
                            name=f"hb{li}_{n}_{h}",
                        )
                        for j in range(h * jh, (h + 1) * jh):
                            ps_h = psh.tile([128, 256], F32, tag="psh")
                            if zform_next:
                                # row-major z via identity transpose:
                                # ps_h[:, co*128:...] = (zT[co][:, j])^T
                                for co in range(mtz):
                                    nc.tensor.matmul(
                                        ps_h[:, co * 128 : (co + 1) * 128],
                                        zt_p[co][:, j * 128 : (j + 1) * 128],
                                        i_sb[:],
                                        start=(co == 0),
                                        stop=(co == mtz - 1),
                                    )
                            else:
                                for kx in range(mtz):
                                    nc.tensor.matmul(
                                        ps_h[:, 0:do2],
                                        zt_p[kx][:, j * 128 : (j + 1) * 128],
                                        w_sb[li + 1][:, kx, :],
                                        start=(kx == 0),
                                        stop=(kx == mtz - 1),
                                    )
                            h_st = hstage.tile([128, 256], BF16, tag="hst")
                            nc.vector.tensor_copy(
                                h_st[:, 0:do2], ps_h[:, 0:do2]
                            )
                            # ACT ring: fires immediately (never queued
                            # behind the sync ring's stream prefetch)
                            nc.scalar.dma_start(
                                bounce[(j - h * jh) * 128
                                       : (j - h * jh + 1) * 128, :],
                                h_st[:, 0:do2],
                            )
                        gath = dram.tile(
                            [NCORES * jh * 128, do2], BF16,
                            addr_space="Shared",
                            tag=f"hg{li}_{n}_{h}", name=f"hg{li}_{n}_{h}",
                        )
                        nc.gpsimd.collective_compute(
                            "AllGather",
                            mybir.AluOpType.bypass,
                            ins=[bounce[:].opt()],
                            outs=[gath[:].opt()],
                            replica_groups=[list(range(NCORES))],
                        )
                        # gather-dependent loads on the SWDGE (gpsimd) ring
                        # (their AG-wait cannot FIFO-block either HWDGE
                        # ring); per core c so early chunks unblock first
                        g_r = gath.rearrange("(c j p) d -> p (c j) d",
                                             p=128, c=NCORES)
                        for c in range(NCORES):
                            nc.gpsimd.dma_start(
                                c_next[n][:, c * HALF + h * jh
                                          : c * HALF + (h + 1) * jh, 0:do2],
                                g_r[:, c * jh : (c + 1) * jh, :],
                            )

                if li in JOINT:
                    emit_block(0, 0)
                    emit_block(0, 1)
                    emit_block(1, 0)
                    emit_block(1, 1)
                    emit_epilogue(0)
                    emit_epilogue(1)
                else:
                    emit_block(0, 0)
                    insert = li > 0 and mt <= 2
                    if insert:
                        emit_block(0, 1, 0, INSERT)
                    emit_block(1, 0)
                    emit_epilogue(0)
                    emit_block(0, 1, INSERT if insert else 0, None)
                    emit_block(1, 1)
                    emit_epilogue(1)

                if not last:
                    c_read = make_reader(c_next, cw_next)

    nc.compile()
    return nc


def make_in_maps(inputs):
    X = np.asarray(inputs["X"], dtype=np.float32)
    adj = np.asarray(inputs["adj_"], dtype=np.float32)
    ws = [np.asarray(inputs[f"W{j + 1}"], np.float32).astype(NP_BF16)
          for j in range(6)]
    # H1 = X @ W1 on the host (0.008% of total FLOPs): removes the xT
    # stream and the XW1 tensor block from the device's critical startup
    h1 = (X @ np.asarray(inputs["W1"], dtype=np.float32)).astype(NP_BF16)
    eye = np.eye(128, dtype=NP_BF16)
    in_maps = []
    for i in range(NCORES):
        rows = slice(i * R, (i + 1) * R)
        m = {
            "adjT": np.ascontiguousarray(adj[rows, :].T).astype(NP_BF16),
            "H1": h1,
            "I128": eye,
        }
        for j in range(6):
            m[f"W{j + 1}"] = ws[j]
        in_maps.append(m)
    return in_maps


def kernel(**inputs):
    if "nc" not in _CACHED:
        _CACHED["nc"] = _build()
    nc = _CACHED["nc"]

    res = run_bass_kernel_spmd(nc, make_in_maps(inputs),
                               core_ids=list(range(NCORES)))
    out = np.concatenate(
        [np.asarray(r["outT"], dtype=np.float32).T for r in res.results], axis=0
    )
    return out

